# revision 25
# baseline (speedup 1.0000x reference)
"""Trainium2 kernel for nn_EncoderTreeSpanNN — split-table single-row gathers.

Design (final tuned version):
- Tables stored as [V, 3*D] f16 (hop-interleaved rows, 768B); each gather
  descriptor fetches only the needed row. Each group's 1024 tokens split
  into EXACTLY 512 lo + 512 hi via overlapping table views (lo=tab[0:32768]
  idx=t, hi=tab[17232:50000] idx=t-17232, both int16-safe); tokens in the
  overlap band are assigned to whichever side balances the split. Zero
  padding, zero memsets, 8 gather blocks per group.
- 24 gather calls round-robin the 4 SWDGE queues starting at queue 1 (the
  library-forcing dummy occupies queue 0, so the first real call is not
  stuck behind its drain). One dedicated gather tile per group: gathers
  never throttle on compute consuming earlier groups.
- All 12 selection matrices (slot -> span routing via is_equal against an
  uploaded iota) are built on DVE during the ~10us GPSIMD library-overlay
  load when DVE is otherwise idle; span reduction is 8 accumulating
  matmuls per group.
- Attention is computed transposed (attT[k,l] = kfT^T cfT) incrementally
  as each KB group's features land: exp (scaled by 2^-4 for f16 range),
  row-sum via ones-vector matmul, and the output matmuls accumulate into
  per-(hop,gg) PSUM regions with per-kk SBUF accumulation. Only the last
  KB block's chain remains in the tail.
"""

import sys

sys.path.insert(0, "/opt/trn_rl_repo")

import numpy as np

import concourse.bacc as bacc
import concourse.tile as tile
from concourse import mybir
from concourse.bass_utils import run_bass_kernel_spmd

# problem constants
V, D, HOPS = 50000, 128, 3
B, Lc, Mc = 16, 256, 8
Lk, Mk = 512, 8
NCORES = 8
BPC = B // NCORES
E3 = HOPS * D  # 384 elems per row (hop-interleaved)
LO_MAX = 32768  # lo view = tab[0:32768], idx = t (int16 max)
HI_BASE = 17232  # hi view = tab[17232:50000], idx = t-17232 (max 32767)
CONV_G = Lc // 128  # 2
KB_G = Lk // 128  # 4
TB = 8  # gather blocks per group (4 lo + 4 hi, exactly 512 tokens each)
NG_PER_B = CONV_G + KB_G  # 6
NG = BPC * NG_PER_B  # 12

F32 = mybir.dt.float32
F16 = mybir.dt.float16
I16 = mybir.dt.int16

EXP_BIAS = float(-4.0 * np.log(2.0))  # exp(att)*2^-4 keeps row-sums in f16 range

# per-core group list: per batch, conv groups then kb groups
GROUPS = []
for _b in range(BPC):
    for _gg in range(CONV_G):
        GROUPS.append(("c", _b, _gg))
    for _gg in range(KB_G):
        GROUPS.append(("k", _b, _gg))


def _pack_idx(flat):
    """[n] int16 -> [128, n//16] dma_gather index layout (8 replicas x 16)."""
    n = flat.shape[0]
    return np.tile(flat.reshape(n // 16, 16).T.astype(np.int16), (8, 1))


def prepare(conv_seqs, kb_arr, C, K):
    conv_seqs = np.asarray(conv_seqs)
    kb_arr = np.asarray(kb_arr)

    def row_table(T):
        # [HOPS, V, D] -> [V, HOPS*D] f16 (hop-interleaved rows)
        return (
            np.transpose(np.asarray(T, np.float32), (1, 0, 2))
            .reshape(V, E3)
            .astype(np.float16)
        )

    tab_c = row_table(C)
    tab_k = row_table(K)

    # per (core, group) split into exactly 512 lo + 512 hi tokens: the two
    # table views overlap on rows [17232, 32768), so tokens in that band can
    # be assigned to either side to balance the counts exactly (zero padding,
    # 8 gather blocks per group instead of 10)
    sides = {}
    for c in range(NCORES):
        for g, (t, b, gg) in enumerate(GROUPS):
            seqs = conv_seqs if t == "c" else kb_arr
            arr = seqs[c * BPC + b, gg * 128 : (gg + 1) * 128, :]  # [128, M]
            toks = arr.reshape(-1).astype(np.int64)  # position p*M+m -> span p
            spans = np.repeat(np.arange(128), arr.shape[1])
            mand_lo = toks < HI_BASE
            flex = (toks >= HI_BASE) & (toks < LO_MAX)
            need = 512 - int(mand_lo.sum())
            assert 0 <= need <= int(flex.sum()), "overlap band can't balance"
            to_lo = mand_lo.copy()
            to_lo[np.nonzero(flex)[0][:need]] = True
            sides[(c, g, 0)] = (toks[to_lo].astype(np.int16), spans[to_lo])
            sides[(c, g, 1)] = (
                (toks[~to_lo] - HI_BASE).astype(np.int16),
                spans[~to_lo],
            )
    cap = 512

    in_maps = []
    for c in range(NCORES):
        # pair-call m = pair p (groups 2p, 2p+1) x side s: 1024 indices at
        # cols m*64..m*64+64 (group 2p's 512 then group 2p+1's 512)
        idx_all = np.empty((128, NG * 2 * (cap // 16)), np.int16)
        seg_all = np.full((128, NG, TB), -1.0, np.float32)
        for g in range(NG):
            for side in range(2):
                sidx, sspan = sides[(c, g, side)]
                n = len(sidx)
                assert n == cap
                m = (g // 2) * 2 + side
                col = m * 64 + (g % 2) * 32
                idx_all[:, col : col + cap // 16] = _pack_idx(sidx)
                # segment ids: slot i (block i//128, partition i%128) -> span
                blk = side * 4 + np.arange(n) // 128
                part = np.arange(n) % 128
                seg_all[part, g, blk] = sspan
        in_maps.append(
            {
                "tab_c": tab_c,
                "tab_k": tab_k,
                "idx_all": idx_all,
                "seg_all": seg_all,
                "ident": np.eye(128, dtype=np.float16),
                "iota": np.broadcast_to(
                    np.arange(128, dtype=np.float32), (128, 128)
                ).copy(),
            }
        )
    return {"cap": cap}, in_maps


def build_nc(meta):
    cap = meta["cap"]
    nc = bacc.Bacc(num_swdge_queues=4)
    tab_c = nc.declare_dram_parameter("tab_c", [V, E3], F16, False)
    tab_k = nc.declare_dram_parameter("tab_k", [V, E3], F16, False)
    idx_d = nc.declare_dram_parameter("idx_all", [128, NG * 2 * (cap // 16)], I16, False)
    seg_d = nc.declare_dram_parameter("seg_all", [128, NG, TB], F32, False)
    ident_d = nc.declare_dram_parameter("ident", [128, 128], F16, False)
    iota_d = nc.declare_dram_parameter("iota", [128, 128], F32, False)
    out_d = nc.declare_dram_parameter("out", [BPC, Lc, D], F32, True)

    tab_lo = {"c": tab_c[0:LO_MAX], "k": tab_k[0:LO_MAX]}
    tab_hi = {"c": tab_c[HI_BASE:V], "k": tab_k[HI_BASE:V]}

    with tile.TileContext(nc) as tc:
        with (
            tc.tile_pool(name="constp", bufs=1) as constp,
            tc.tile_pool(name="gp", bufs=1) as gp,
            tc.tile_pool(name="sp", bufs=1) as sp,
            tc.tile_pool(name="featp", bufs=1) as featp,
            tc.tile_pool(name="expp", bufs=3) as expp,
            tc.tile_pool(name="softp", bufs=4) as softp,
            tc.tile_pool(name="spanps_p", bufs=2, space="PSUM") as spanps_p,
            tc.tile_pool(name="attps_p", bufs=2, space="PSUM") as attps_p,
            tc.tile_pool(name="tp_p", bufs=2, space="PSUM") as tp_p,
            tc.tile_pool(name="outps_p", bufs=1, space="PSUM") as outps_p,
        ):
            # tiny dummy gather first: forces the GPSIMD library load (~9us
            # of DMA residency) to start before the input uploads
            dummy_idx = constp.tile([128, 1], I16)
            nc.vector.memset(dummy_idx[:], 0)
            dummy_out = constp.tile([128, 1, E3], F16)
            nc.gpsimd.dma_gather(
                out_ap=dummy_out[:],
                in_ap=tab_c[0:LO_MAX],
                idxs_ap=dummy_idx[:],
                num_idxs=16,
                num_idxs_reg=16,
                elem_size=E3,
                queue_num=0,
            )
            idx_sb = constp.tile([128, NG * 2 * (cap // 16)], I16)
            nc.sync.dma_start(out=idx_sb[:], in_=idx_d[:])
            seg_sb = constp.tile([128, NG, TB], F32)
            nc.sync.dma_start(out=seg_sb[:], in_=seg_d[:])
            ident = constp.tile([128, 128], F16)
            nc.sync.dma_start(out=ident[:], in_=ident_d[:])
            iota = constp.tile([128, 128], F32)
            nc.sync.dma_start(out=iota[:], in_=iota_d[:])
            ones128 = constp.tile([128, 1], F16)
            nc.vector.memset(ones128[:], 1.0)
            ebias = constp.tile([128, 1], F32)
            nc.vector.memset(ebias[:], EXP_BIAS)

            cf3 = [
                featp.tile([128, CONV_G, HOPS, D], F16, name=f"cf3_{b}")
                for b in range(BPC)
            ]
            kf3 = [
                featp.tile([128, KB_G, HOPS, D], F16, name=f"kf3_{b}")
                for b in range(BPC)
            ]
            cfT3 = [
                featp.tile([128, HOPS, Lc], F16, name=f"cfT3_{b}") for b in range(BPC)
            ]
            kfT3 = [
                featp.tile([128, HOPS, Lk], F16, name=f"kfT3_{b}") for b in range(BPC)
            ]
            oacc = [
                featp.tile([128, CONV_G, D], F32, name=f"oacc_{b}") for b in range(BPC)
            ]


            # selection matrices prebuilt during the GPSIMD library-load
            # window (DVE is otherwise idle then)
            s_tiles = []
            for g in range(NG):
                s_g = sp.tile([128, TB, 128], F16, name=f"S_{g}")
                nc.vector.tensor_tensor(
                    out=s_g[:],
                    in0=seg_sb[:, g, :]
                    .rearrange("p (t o) -> p t o", o=1)
                    .to_broadcast([128, TB, 128]),
                    in1=iota[:]
                    .rearrange("p (o d) -> p o d", o=1)
                    .to_broadcast([128, TB, 128]),
                    op=mybir.AluOpType.is_equal,
                )
                s_tiles.append(s_g)

            # pair tiles [128, side, group-in-pair, blk, elem]; each gather
            # call covers BOTH groups of a pair on one side (1024 rows), so
            # each SWDGE queue serves 3 calls instead of 6 — half the
            # gen/semaphore bubbles between a queue's batches
            pt_tiles = [
                gp.tile([128, 2, 2, 4, E3], F16, name=f"pt_{p}")
                for p in range(NG // 2)
            ]
            for p in range(NG // 2):
                t = GROUPS[2 * p][0]
                assert t == GROUPS[2 * p + 1][0]
                for side, tabs in ((0, tab_lo), (1, tab_hi)):
                    m = 2 * p + side
                    nc.gpsimd.dma_gather(
                        out_ap=pt_tiles[p][:, side].rearrange(
                            "p a b e -> p (a b) e"
                        ),
                        in_ap=tabs[t][:],
                        idxs_ap=idx_sb[:, m * 64 : m * 64 + 64],
                        num_idxs=2 * cap,
                        num_idxs_reg=2 * cap,
                        elem_size=E3,
                        queue_num=(1 + m) % 4,
                    )

            def do_group(g):
                t, b, gg = GROUPS[g]
                pt = pt_tiles[g // 2]
                gw = g % 2
                s_g = s_tiles[g]
                ps = spanps_p.tile([128, E3], F32, tag="ps", name=f"ps_{g}")
                for blk in range(TB):
                    nc.tensor.matmul(
                        out=ps[:],
                        lhsT=s_g[:, blk, :],
                        rhs=pt[:, blk // 4, gw, blk % 4, :],
                        start=(blk == 0),
                        stop=(blk == TB - 1),
                    )
                feat = cf3[b] if t == "c" else kf3[b]
                nc.vector.tensor_copy(out=feat[:, gg, :, :], in_=ps[:])
                featT = cfT3[b] if t == "c" else kfT3[b]
                tp = tp_p.tile([128, HOPS, 128], F16, tag="tp", name=f"tpg_{g}")
                for hop in range(HOPS):
                    nc.tensor.transpose(
                        out=tp[:, hop, :], in_=feat[:, gg, hop, :], identity=ident[:]
                    )
                nc.vector.tensor_copy(
                    out=featT[:, :, gg * 128 : (gg + 1) * 128], in_=tp[:]
                )

            def do_att_incr(b, kk, acc):
                # part: six bank-aligned 128-f32 output regions at (hop*2+gg)
                # *128, then six transposed softmax row-sums at 768+. Every
                # matmul is its own start+stop accumulation group — PSUM
                # corrupts when several open groups share a bank — and the
                # cross-KB-block accumulation happens in SBUF (acc) instead.
                part = outps_p.tile([128, 1024], F32, tag="part", name=f"pt_{b}_{kk}")
                for hop in range(HOPS):
                    att = attps_p.tile(
                        [128, Lc], F32, tag="att", name=f"att_{b}_{kk}_{hop}"
                    )
                    nc.tensor.matmul(
                        out=att[:],
                        lhsT=kfT3[b][:, hop, kk * 128 : (kk + 1) * 128],
                        rhs=cfT3[b][:, hop, :],
                        start=True,
                        stop=True,
                    )
                    expT = expp.tile(
                        [128, Lc], F16, tag="expT", name=f"exp_{b}_{kk}_{hop}"
                    )
                    nc.scalar.activation(
                        out=expT[:],
                        in_=att[:],
                        func=mybir.ActivationFunctionType.Exp,
                        bias=ebias[:],
                    )
                    for gg in range(CONV_G):
                        r = hop * CONV_G + gg
                        # row-sums land transposed: rsum[l, 1] = expT[:, l].T @ 1
                        nc.tensor.matmul(
                            out=part[:, 768 + r : 769 + r],
                            lhsT=expT[:, gg * 128 : (gg + 1) * 128],
                            rhs=ones128[:],
                            start=True,
                            stop=True,
                        )
                        nc.tensor.matmul(
                            out=part[:, r * D : (r + 1) * D],
                            lhsT=expT[:, gg * 128 : (gg + 1) * 128],
                            rhs=kf3[b][:, kk, hop, :],
                            start=True,
                            stop=True,
                        )
                if kk == 0:
                    nc.vector.tensor_copy(out=acc[:], in_=part[:, 0:774])
                else:
                    nc.vector.tensor_add(out=acc[:], in0=acc[:], in1=part[:, 0:774])

            def finalize(b, acc):
                rinv = softp.tile([128, HOPS, CONV_G], F32, tag="rinv_sb", name=f"ri_{b}")
                nc.vector.reciprocal(
                    out=rinv[:],
                    in_=acc[:, 768 : 768 + HOPS * CONV_G].rearrange(
                        "p (h g) -> p h g", h=HOPS
                    ),
                )
                # scale all six regions by 1/rsum in one broadcast multiply,
                # then fold the three hops with two adds
                sc = softp.tile([128, HOPS, CONV_G, D], F32, tag="sc", name=f"sc_{b}")
                nc.vector.tensor_tensor(
                    out=sc[:],
                    in0=acc[:, 0 : HOPS * CONV_G * D].rearrange(
                        "p (h g d) -> p h g d", h=HOPS, g=CONV_G
                    ),
                    in1=rinv[:]
                    .rearrange("p h (g o) -> p h g o", o=1)
                    .to_broadcast([128, HOPS, CONV_G, D]),
                    op=mybir.AluOpType.mult,
                )
                nc.vector.tensor_add(
                    out=oacc[b][:], in0=sc[:, 0], in1=sc[:, 1]
                )
                nc.vector.tensor_add(
                    out=oacc[b][:], in0=oacc[b][:], in1=sc[:, 2]
                )
                for gg in range(CONV_G):
                    nc.sync.dma_start(
                        out=out_d[b, gg * 128 : (gg + 1) * 128, :], in_=oacc[b][:, gg, :]
                    )

            accs = [featp.tile([128, 774], F32, name=f"acc_{b}") for b in range(BPC)]
            kb_seen = [0] * BPC
            for g, (t, b, gg) in enumerate(GROUPS):
                do_group(g)
                if t == "k":
                    do_att_incr(b, gg, accs[b])
                    kb_seen[b] += 1
                    if kb_seen[b] == KB_G:
                        finalize(b, accs[b])
    nc.compile()
    return nc


def assemble_output(results):
    out = np.empty((Lc, B, D), np.float32)
    for c in range(NCORES):
        o = results[c]["out"]
        for b in range(BPC):
            out[:, c * BPC + b, :] = o[b]
    return out


def kernel(conv_seqs, kb_arr, C, K):
    meta, in_maps = prepare(conv_seqs, kb_arr, C, K)
    nc = build_nc(meta)
    res = run_bass_kernel_spmd(nc, in_maps, list(range(NCORES))).results
    return assemble_output(res)



# revision 27
# speedup vs baseline: 1.1043x; 1.1043x over previous
"""Trainium2 kernel for nn_EncoderTreeSpanNN — split-table single-row gathers.

Design (final tuned version):
- Tables stored as [V, 3*D] f16 (hop-interleaved rows, 768B); each gather
  descriptor fetches only the needed row. Each group's 1024 tokens split
  into EXACTLY 512 lo + 512 hi via overlapping table views (lo=tab[0:32768]
  idx=t, hi=tab[17232:50000] idx=t-17232, both int16-safe); tokens in the
  overlap band are assigned to whichever side balances the split. Zero
  padding, zero memsets, 8 gather blocks per group.
- 24 gather calls round-robin the 4 SWDGE queues starting at queue 1 (the
  library-forcing dummy occupies queue 0, so the first real call is not
  stuck behind its drain). One dedicated gather tile per group: gathers
  never throttle on compute consuming earlier groups.
- All 12 selection matrices (slot -> span routing via is_equal against an
  uploaded iota) are built on DVE during the ~10us GPSIMD library-overlay
  load when DVE is otherwise idle; span reduction is 8 accumulating
  matmuls per group.
- Attention is computed transposed (attT[k,l] = kfT^T cfT) incrementally
  as each KB group's features land: exp (scaled by 2^-4 for f16 range),
  row-sum via ones-vector matmul, and the output matmuls accumulate into
  per-(hop,gg) PSUM regions with per-kk SBUF accumulation. Only the last
  KB block's chain remains in the tail.
"""

import sys

sys.path.insert(0, "/opt/trn_rl_repo")

import numpy as np

import concourse.bacc as bacc
import concourse.tile as tile
from concourse import mybir
from concourse.bass_utils import run_bass_kernel_spmd

# problem constants
V, D, HOPS = 50000, 128, 3
B, Lc, Mc = 16, 256, 8
Lk, Mk = 512, 8
NCORES = 8
BPC = B // NCORES
E3 = HOPS * D  # 384 elems per row (hop-interleaved)
LO_MAX = 32768  # lo view = tab[0:32768], idx = t (int16 max)
HI_BASE = 17232  # hi view = tab[17232:50000], idx = t-17232 (max 32767)
CONV_G = Lc // 128  # 2
KB_G = Lk // 128  # 4
TB = 8  # gather blocks per group (4 lo + 4 hi, exactly 512 tokens each)
NG_PER_B = CONV_G + KB_G  # 6
NG = BPC * NG_PER_B  # 12

F32 = mybir.dt.float32
F16 = mybir.dt.float16
I16 = mybir.dt.int16

EXP_BIAS = float(-4.0 * np.log(2.0))  # exp(att)*2^-4 keeps row-sums in f16 range

# per-core group list: per batch, conv groups then kb groups
GROUPS = []
for _b in range(BPC):
    for _gg in range(CONV_G):
        GROUPS.append(("c", _b, _gg))
    for _gg in range(KB_G):
        GROUPS.append(("k", _b, _gg))


def _pack_idx(flat):
    """[n] int16 -> [128, n//16] dma_gather index layout (8 replicas x 16)."""
    n = flat.shape[0]
    return np.tile(flat.reshape(n // 16, 16).T.astype(np.int16), (8, 1))


def prepare(conv_seqs, kb_arr, C, K):
    conv_seqs = np.asarray(conv_seqs)
    kb_arr = np.asarray(kb_arr)

    def row_table(T):
        # [HOPS, V, D] -> [V, HOPS*D] f16 (hop-interleaved rows)
        return (
            np.transpose(np.asarray(T, np.float32), (1, 0, 2))
            .reshape(V, E3)
            .astype(np.float16)
        )

    tab_c = row_table(C)
    tab_k = row_table(K)

    # per (core, group) split into exactly 512 lo + 512 hi tokens: the two
    # table views overlap on rows [17232, 32768), so tokens in that band can
    # be assigned to either side to balance the counts exactly (zero padding,
    # 8 gather blocks per group instead of 10)
    sides = {}
    for c in range(NCORES):
        for g, (t, b, gg) in enumerate(GROUPS):
            seqs = conv_seqs if t == "c" else kb_arr
            arr = seqs[c * BPC + b, gg * 128 : (gg + 1) * 128, :]  # [128, M]
            toks = arr.reshape(-1).astype(np.int64)  # position p*M+m -> span p
            spans = np.repeat(np.arange(128), arr.shape[1])
            mand_lo = toks < HI_BASE
            flex = (toks >= HI_BASE) & (toks < LO_MAX)
            need = 512 - int(mand_lo.sum())
            assert 0 <= need <= int(flex.sum()), "overlap band can't balance"
            to_lo = mand_lo.copy()
            to_lo[np.nonzero(flex)[0][:need]] = True
            sides[(c, g, 0)] = (toks[to_lo].astype(np.int16), spans[to_lo])
            sides[(c, g, 1)] = (
                (toks[~to_lo] - HI_BASE).astype(np.int16),
                spans[~to_lo],
            )
    cap = 512

    in_maps = []
    for c in range(NCORES):
        idx_all = np.empty((128, NG * 2 * (cap // 16)), np.int16)
        seg_all = np.full((128, NG, TB), -1.0, np.float32)
        for g in range(NG):
            for side in range(2):
                sidx, sspan = sides[(c, g, side)]
                n = len(sidx)
                assert n == cap
                col = (g * 2 + side) * (cap // 16)
                idx_all[:, col : col + cap // 16] = _pack_idx(sidx)
                # segment ids: slot i (block i//128, partition i%128) -> span
                blk = side * 4 + np.arange(n) // 128
                part = np.arange(n) % 128
                seg_all[part, g, blk] = sspan
        in_maps.append(
            {
                "tab_c": tab_c,
                "tab_k": tab_k,
                "idx_all": idx_all,
                "seg_all": seg_all,
                "ident": np.eye(128, dtype=np.float16),
                "iota": np.broadcast_to(
                    np.arange(128, dtype=np.float32), (128, 128)
                ).copy(),
            }
        )
    return {"cap": cap}, in_maps


def build_nc(meta):
    cap = meta["cap"]
    nc = bacc.Bacc(num_swdge_queues=4)
    tab_c = nc.declare_dram_parameter("tab_c", [V, E3], F16, False)
    tab_k = nc.declare_dram_parameter("tab_k", [V, E3], F16, False)
    idx_d = nc.declare_dram_parameter("idx_all", [128, NG * 2 * (cap // 16)], I16, False)
    seg_d = nc.declare_dram_parameter("seg_all", [128, NG, TB], F32, False)
    ident_d = nc.declare_dram_parameter("ident", [128, 128], F16, False)
    iota_d = nc.declare_dram_parameter("iota", [128, 128], F32, False)
    out_d = nc.declare_dram_parameter("out", [BPC, Lc, D], F32, True)

    tab_lo = {"c": tab_c[0:LO_MAX], "k": tab_k[0:LO_MAX]}
    tab_hi = {"c": tab_c[HI_BASE:V], "k": tab_k[HI_BASE:V]}

    with tile.TileContext(nc) as tc:
        with (
            tc.tile_pool(name="constp", bufs=1) as constp,
            tc.tile_pool(name="gp", bufs=1) as gp,
            tc.tile_pool(name="sp", bufs=1) as sp,
            tc.tile_pool(name="featp", bufs=1) as featp,
            tc.tile_pool(name="expp", bufs=3) as expp,
            tc.tile_pool(name="softp", bufs=4) as softp,
            tc.tile_pool(name="spanps_p", bufs=2, space="PSUM") as spanps_p,
            tc.tile_pool(name="attps_p", bufs=2, space="PSUM") as attps_p,
            tc.tile_pool(name="tp_p", bufs=2, space="PSUM") as tp_p,
            tc.tile_pool(name="outps_p", bufs=1, space="PSUM") as outps_p,
        ):
            # tiny dummy gather first: forces the GPSIMD library load (~9us
            # of DMA residency) to start before the input uploads
            dummy_idx = constp.tile([128, 1], I16)
            nc.vector.memset(dummy_idx[:], 0)
            dummy_out = constp.tile([128, 1, E3], F16)
            nc.gpsimd.dma_gather(
                out_ap=dummy_out[:],
                in_ap=tab_c[0:LO_MAX],
                idxs_ap=dummy_idx[:],
                num_idxs=16,
                num_idxs_reg=16,
                elem_size=E3,
                queue_num=0,
            )
            idx_sb = constp.tile([128, NG * 2 * (cap // 16)], I16)
            nc.sync.dma_start(out=idx_sb[:], in_=idx_d[:])
            seg_sb = constp.tile([128, NG, TB], F32)
            nc.sync.dma_start(out=seg_sb[:], in_=seg_d[:])
            ident = constp.tile([128, 128], F16)
            nc.sync.dma_start(out=ident[:], in_=ident_d[:])
            iota = constp.tile([128, 128], F32)
            nc.sync.dma_start(out=iota[:], in_=iota_d[:])
            ones128 = constp.tile([128, 1], F16)
            nc.vector.memset(ones128[:], 1.0)
            ebias = constp.tile([128, 1], F32)
            nc.vector.memset(ebias[:], EXP_BIAS)

            cf3 = [
                featp.tile([128, CONV_G, HOPS, D], F16, name=f"cf3_{b}")
                for b in range(BPC)
            ]
            kf3 = [
                featp.tile([128, KB_G, HOPS, D], F16, name=f"kf3_{b}")
                for b in range(BPC)
            ]
            cfT3 = [
                featp.tile([128, HOPS, Lc], F16, name=f"cfT3_{b}") for b in range(BPC)
            ]
            kfT3 = [
                featp.tile([128, HOPS, Lk], F16, name=f"kfT3_{b}") for b in range(BPC)
            ]
            oacc = [
                featp.tile([128, CONV_G, D], F32, name=f"oacc_{b}") for b in range(BPC)
            ]

            qctr = [1]  # first real gather on q1: q0 is busy with the dummy

            # selection matrices prebuilt during the GPSIMD library-load
            # window (DVE is otherwise idle then)
            s_tiles = []
            for g in range(NG):
                s_g = sp.tile([128, TB, 128], F16, name=f"S_{g}")
                nc.vector.tensor_tensor(
                    out=s_g[:],
                    in0=seg_sb[:, g, :]
                    .rearrange("p (t o) -> p t o", o=1)
                    .to_broadcast([128, TB, 128]),
                    in1=iota[:]
                    .rearrange("p (o d) -> p o d", o=1)
                    .to_broadcast([128, TB, 128]),
                    op=mybir.AluOpType.is_equal,
                )
                s_tiles.append(s_g)

            def do_group(g):
                t, b, gg = GROUPS[g]
                gt = gp.tile([128, TB, E3], F16, tag=f"gt_{g}", name=f"gt_{g}")
                for side, tabs in ((0, tab_lo), (1, tab_hi)):
                    col = (g * 2 + side) * (cap // 16)
                    nc.gpsimd.dma_gather(
                        out_ap=gt[:, side * 4 : side * 4 + 4, :],
                        in_ap=tabs[t][:],
                        idxs_ap=idx_sb[:, col : col + cap // 16],
                        num_idxs=cap,
                        num_idxs_reg=cap,
                        elem_size=E3,
                        queue_num=qctr[0] % 4,
                    )
                    qctr[0] += 1
                s_g = s_tiles[g]
                ps = spanps_p.tile([128, E3], F32, tag="ps", name=f"ps_{g}")
                for blk in range(TB):
                    nc.tensor.matmul(
                        out=ps[:],
                        lhsT=s_g[:, blk, :],
                        rhs=gt[:, blk, :],
                        start=(blk == 0),
                        stop=(blk == TB - 1),
                    )
                feat = cf3[b] if t == "c" else kf3[b]
                # PSUM->SBUF f32->f16 cast on the (mostly idle) scalar engine
                # frees DVE bandwidth during the gather window
                nc.scalar.activation(
                    out=feat[:, gg, :, :],
                    in_=ps[:].rearrange("p (h d) -> p h d", h=HOPS),
                    func=mybir.ActivationFunctionType.Copy,
                )
                featT = cfT3[b] if t == "c" else kfT3[b]
                tp = tp_p.tile([128, HOPS, 128], F16, tag="tp", name=f"tpg_{g}")
                for hop in range(HOPS):
                    nc.tensor.transpose(
                        out=tp[:, hop, :], in_=feat[:, gg, hop, :], identity=ident[:]
                    )
                nc.vector.tensor_copy(
                    out=featT[:, :, gg * 128 : (gg + 1) * 128], in_=tp[:]
                )

            def do_att_incr(b, kk, acc):
                # part: six bank-aligned 128-f32 output regions at (hop*2+gg)
                # *128, then six transposed softmax row-sums at 768+. Every
                # matmul is its own start+stop accumulation group — PSUM
                # corrupts when several open groups share a bank — and the
                # cross-KB-block accumulation happens in SBUF (acc) instead.
                part = outps_p.tile([128, 1024], F32, tag="part", name=f"pt_{b}_{kk}")
                for hop in range(HOPS):
                    att = attps_p.tile(
                        [128, Lc], F32, tag="att", name=f"att_{b}_{kk}_{hop}"
                    )
                    nc.tensor.matmul(
                        out=att[:],
                        lhsT=kfT3[b][:, hop, kk * 128 : (kk + 1) * 128],
                        rhs=cfT3[b][:, hop, :],
                        start=True,
                        stop=True,
                    )
                    expT = expp.tile(
                        [128, Lc], F16, tag="expT", name=f"exp_{b}_{kk}_{hop}"
                    )
                    nc.scalar.activation(
                        out=expT[:],
                        in_=att[:],
                        func=mybir.ActivationFunctionType.Exp,
                        bias=ebias[:],
                    )
                    for gg in range(CONV_G):
                        r = hop * CONV_G + gg
                        # row-sums land transposed: rsum[l, 1] = expT[:, l].T @ 1
                        nc.tensor.matmul(
                            out=part[:, 768 + r : 769 + r],
                            lhsT=expT[:, gg * 128 : (gg + 1) * 128],
                            rhs=ones128[:],
                            start=True,
                            stop=True,
                        )
                        nc.tensor.matmul(
                            out=part[:, r * D : (r + 1) * D],
                            lhsT=expT[:, gg * 128 : (gg + 1) * 128],
                            rhs=kf3[b][:, kk, hop, :],
                            start=True,
                            stop=True,
                        )
                if kk == 0:
                    nc.vector.tensor_copy(out=acc[:], in_=part[:, 0:774])
                else:
                    nc.vector.tensor_add(out=acc[:], in0=acc[:], in1=part[:, 0:774])

            def finalize(b, acc):
                rinv = softp.tile([128, HOPS, CONV_G], F32, tag="rinv_sb", name=f"ri_{b}")
                nc.vector.reciprocal(
                    out=rinv[:],
                    in_=acc[:, 768 : 768 + HOPS * CONV_G].rearrange(
                        "p (h g) -> p h g", h=HOPS
                    ),
                )
                # scale all six regions by 1/rsum in one broadcast multiply,
                # then fold the three hops with two adds
                sc = softp.tile([128, HOPS, CONV_G, D], F32, tag="sc", name=f"sc_{b}")
                nc.vector.tensor_tensor(
                    out=sc[:],
                    in0=acc[:, 0 : HOPS * CONV_G * D].rearrange(
                        "p (h g d) -> p h g d", h=HOPS, g=CONV_G
                    ),
                    in1=rinv[:]
                    .rearrange("p h (g o) -> p h g o", o=1)
                    .to_broadcast([128, HOPS, CONV_G, D]),
                    op=mybir.AluOpType.mult,
                )
                nc.vector.tensor_add(
                    out=oacc[b][:], in0=sc[:, 0], in1=sc[:, 1]
                )
                nc.vector.tensor_add(
                    out=oacc[b][:], in0=oacc[b][:], in1=sc[:, 2]
                )
                for gg in range(CONV_G):
                    nc.sync.dma_start(
                        out=out_d[b, gg * 128 : (gg + 1) * 128, :], in_=oacc[b][:, gg, :]
                    )

            accs = [featp.tile([128, 774], F32, name=f"acc_{b}") for b in range(BPC)]
            kb_seen = [0] * BPC
            for g, (t, b, gg) in enumerate(GROUPS):
                do_group(g)
                if t == "k":
                    do_att_incr(b, gg, accs[b])
                    kb_seen[b] += 1
                    if kb_seen[b] == KB_G:
                        finalize(b, accs[b])
    nc.compile()
    return nc


def assemble_output(results):
    out = np.empty((Lc, B, D), np.float32)
    for c in range(NCORES):
        o = results[c]["out"]
        for b in range(BPC):
            out[:, c * BPC + b, :] = o[b]
    return out


def kernel(conv_seqs, kb_arr, C, K):
    meta, in_maps = prepare(conv_seqs, kb_arr, C, K)
    nc = build_nc(meta)
    res = run_bass_kernel_spmd(nc, in_maps, list(range(NCORES))).results
    return assemble_output(res)



# revision 29
# speedup vs baseline: 1.1114x; 1.0064x over previous
"""Trainium2 kernel for nn_EncoderTreeSpanNN — split-table single-row gathers.

Design (final tuned version):
- Tables stored as [V, 3*D] f16 (hop-interleaved rows, 768B); each gather
  descriptor fetches only the needed row. Each group's 1024 tokens split
  into EXACTLY 512 lo + 512 hi via overlapping table views (lo=tab[0:32768]
  idx=t, hi=tab[17232:50000] idx=t-17232, both int16-safe); tokens in the
  overlap band are assigned to whichever side balances the split. Zero
  padding, zero memsets, 8 gather blocks per group.
- 24 gather calls round-robin the 4 SWDGE queues starting at queue 1 (the
  library-forcing dummy occupies queue 0, so the first real call is not
  stuck behind its drain). One dedicated gather tile per group: gathers
  never throttle on compute consuming earlier groups.
- All 12 selection matrices (slot -> span routing via is_equal against an
  uploaded iota) are built on DVE during the ~10us GPSIMD library-overlay
  load when DVE is otherwise idle; span reduction is 8 accumulating
  matmuls per group.
- Attention is computed transposed (attT[k,l] = kfT^T cfT) incrementally
  as each KB group's features land: exp (scaled by 2^-4 for f16 range),
  row-sum via ones-vector matmul, and the output matmuls accumulate into
  per-(hop,gg) PSUM regions with per-kk SBUF accumulation. Only the last
  KB block's chain remains in the tail.
"""

import sys

sys.path.insert(0, "/opt/trn_rl_repo")

import numpy as np

import concourse.bacc as bacc
import concourse.tile as tile
from concourse import mybir
from concourse.bass_utils import run_bass_kernel_spmd

# problem constants
V, D, HOPS = 50000, 128, 3
B, Lc, Mc = 16, 256, 8
Lk, Mk = 512, 8
NCORES = 8
BPC = B // NCORES
E3 = HOPS * D  # 384 elems per row (hop-interleaved)
LO_MAX = 32768  # lo view = tab[0:32768], idx = t (int16 max)
HI_BASE = 17232  # hi view = tab[17232:50000], idx = t-17232 (max 32767)
CONV_G = Lc // 128  # 2
KB_G = Lk // 128  # 4
TB = 8  # gather blocks per group (4 lo + 4 hi, exactly 512 tokens each)
NG_PER_B = CONV_G + KB_G  # 6
NG = BPC * NG_PER_B  # 12

F32 = mybir.dt.float32
F16 = mybir.dt.float16
I16 = mybir.dt.int16

EXP_BIAS = float(-4.0 * np.log(2.0))  # exp(att)*2^-4 keeps row-sums in f16 range

# per-core group list: per batch, conv groups then kb groups
GROUPS = []
for _b in range(BPC):
    for _gg in range(CONV_G):
        GROUPS.append(("c", _b, _gg))
    for _gg in range(KB_G):
        GROUPS.append(("k", _b, _gg))


def _pack_idx(flat):
    """[n] int16 -> [128, n//16] dma_gather index layout (8 replicas x 16)."""
    n = flat.shape[0]
    return np.tile(flat.reshape(n // 16, 16).T.astype(np.int16), (8, 1))


def prepare(conv_seqs, kb_arr, C, K):
    conv_seqs = np.asarray(conv_seqs)
    kb_arr = np.asarray(kb_arr)

    def row_table(T):
        # [HOPS, V, D] -> [V, HOPS*D] f16 (hop-interleaved rows)
        return (
            np.transpose(np.asarray(T, np.float32), (1, 0, 2))
            .reshape(V, E3)
            .astype(np.float16)
        )

    tab_c = row_table(C)
    tab_k = row_table(K)

    # per (core, group) split into exactly 512 lo + 512 hi tokens: the two
    # table views overlap on rows [17232, 32768), so tokens in that band can
    # be assigned to either side to balance the counts exactly (zero padding,
    # 8 gather blocks per group instead of 10)
    sides = {}
    for c in range(NCORES):
        for g, (t, b, gg) in enumerate(GROUPS):
            seqs = conv_seqs if t == "c" else kb_arr
            arr = seqs[c * BPC + b, gg * 128 : (gg + 1) * 128, :]  # [128, M]
            toks = arr.reshape(-1).astype(np.int64)  # position p*M+m -> span p
            spans = np.repeat(np.arange(128), arr.shape[1])
            mand_lo = toks < HI_BASE
            flex = (toks >= HI_BASE) & (toks < LO_MAX)
            need = 512 - int(mand_lo.sum())
            assert 0 <= need <= int(flex.sum()), "overlap band can't balance"
            to_lo = mand_lo.copy()
            to_lo[np.nonzero(flex)[0][:need]] = True
            sides[(c, g, 0)] = (toks[to_lo].astype(np.int16), spans[to_lo])
            sides[(c, g, 1)] = (
                (toks[~to_lo] - HI_BASE).astype(np.int16),
                spans[~to_lo],
            )
    cap = 512

    in_maps = []
    for c in range(NCORES):
        idx_all = np.empty((128, NG * 2 * (cap // 16)), np.int16)
        seg_all = np.full((128, NG, TB), -1.0, np.float32)
        for g in range(NG):
            for side in range(2):
                sidx, sspan = sides[(c, g, side)]
                n = len(sidx)
                assert n == cap
                col = (g * 2 + side) * (cap // 16)
                idx_all[:, col : col + cap // 16] = _pack_idx(sidx)
                # segment ids: slot i (block i//128, partition i%128) -> span
                blk = side * 4 + np.arange(n) // 128
                part = np.arange(n) % 128
                seg_all[part, g, blk] = sspan
        in_maps.append(
            {
                "tab_c": tab_c,
                "tab_k": tab_k,
                "idx_all": idx_all,
                "seg_all": seg_all,
                "ident": np.eye(128, dtype=np.float16),
                "iota": np.broadcast_to(
                    np.arange(128, dtype=np.float32), (128, 128)
                ).copy(),
            }
        )
    return {"cap": cap}, in_maps


def build_nc(meta):
    cap = meta["cap"]
    nc = bacc.Bacc(num_swdge_queues=4)
    tab_c = nc.declare_dram_parameter("tab_c", [V, E3], F16, False)
    tab_k = nc.declare_dram_parameter("tab_k", [V, E3], F16, False)
    idx_d = nc.declare_dram_parameter("idx_all", [128, NG * 2 * (cap // 16)], I16, False)
    seg_d = nc.declare_dram_parameter("seg_all", [128, NG, TB], F32, False)
    ident_d = nc.declare_dram_parameter("ident", [128, 128], F16, False)
    iota_d = nc.declare_dram_parameter("iota", [128, 128], F32, False)
    out_d = nc.declare_dram_parameter("out", [BPC, Lc, D], F32, True)

    tab_lo = {"c": tab_c[0:LO_MAX], "k": tab_k[0:LO_MAX]}
    tab_hi = {"c": tab_c[HI_BASE:V], "k": tab_k[HI_BASE:V]}

    with tile.TileContext(nc) as tc:
        with (
            tc.tile_pool(name="constp", bufs=1) as constp,
            tc.tile_pool(name="gp", bufs=1) as gp,
            tc.tile_pool(name="sp", bufs=1) as sp,
            tc.tile_pool(name="featp", bufs=1) as featp,
            tc.tile_pool(name="expp", bufs=3) as expp,
            tc.tile_pool(name="softp", bufs=4) as softp,
            tc.tile_pool(name="spanps_p", bufs=2, space="PSUM") as spanps_p,
            tc.tile_pool(name="attps_p", bufs=2, space="PSUM") as attps_p,
            tc.tile_pool(name="tp_p", bufs=2, space="PSUM") as tp_p,
            tc.tile_pool(name="outps_p", bufs=1, space="PSUM") as outps_p,
        ):
            # tiny dummy gather first: forces the GPSIMD library load (~9us
            # of DMA residency) to start before the input uploads
            dummy_idx = constp.tile([128, 1], I16)
            nc.vector.memset(dummy_idx[:], 0)
            dummy_out = constp.tile([128, 1, E3], F16)
            nc.gpsimd.dma_gather(
                out_ap=dummy_out[:],
                in_ap=tab_c[0:LO_MAX],
                idxs_ap=dummy_idx[:],
                num_idxs=16,
                num_idxs_reg=16,
                elem_size=E3,
                queue_num=0,
            )
            idx_sb = constp.tile([128, NG * 2 * (cap // 16)], I16)
            nc.sync.dma_start(out=idx_sb[:], in_=idx_d[:])
            seg_sb = constp.tile([128, NG, TB], F32)
            nc.sync.dma_start(out=seg_sb[:], in_=seg_d[:])
            ident = constp.tile([128, 128], F16)
            nc.sync.dma_start(out=ident[:], in_=ident_d[:])
            iota = constp.tile([128, 128], F32)
            nc.sync.dma_start(out=iota[:], in_=iota_d[:])
            ones128 = constp.tile([128, 1], F16)
            nc.vector.memset(ones128[:], 1.0)
            ebias = constp.tile([128, 1], F32)
            nc.vector.memset(ebias[:], EXP_BIAS)

            cf3 = [
                featp.tile([128, CONV_G, HOPS, D], F16, name=f"cf3_{b}")
                for b in range(BPC)
            ]
            kf3 = [
                featp.tile([128, KB_G, HOPS, D], F16, name=f"kf3_{b}")
                for b in range(BPC)
            ]
            cfT3 = [
                featp.tile([128, HOPS, Lc], F16, name=f"cfT3_{b}") for b in range(BPC)
            ]
            kfT3 = [
                featp.tile([128, HOPS, Lk], F16, name=f"kfT3_{b}") for b in range(BPC)
            ]
            oacc = [
                featp.tile([128, CONV_G, D], F32, name=f"oacc_{b}") for b in range(BPC)
            ]

            qctr = [1]  # first real gather on q1: q0 is busy with the dummy

            # selection matrices prebuilt during the GPSIMD library-load
            # window (DVE is otherwise idle then)
            s_tiles = []
            for g in range(NG):
                s_g = sp.tile([128, TB, 128], F16, name=f"S_{g}")
                nc.vector.tensor_tensor(
                    out=s_g[:],
                    in0=seg_sb[:, g, :]
                    .rearrange("p (t o) -> p t o", o=1)
                    .to_broadcast([128, TB, 128]),
                    in1=iota[:]
                    .rearrange("p (o d) -> p o d", o=1)
                    .to_broadcast([128, TB, 128]),
                    op=mybir.AluOpType.is_equal,
                )
                s_tiles.append(s_g)

            def do_group(g):
                t, b, gg = GROUPS[g]
                gt = gp.tile([128, TB, E3], F16, tag=f"gt_{g}", name=f"gt_{g}")
                for side, tabs in ((0, tab_lo), (1, tab_hi)):
                    col = (g * 2 + side) * (cap // 16)
                    nc.gpsimd.dma_gather(
                        out_ap=gt[:, side * 4 : side * 4 + 4, :],
                        in_ap=tabs[t][:],
                        idxs_ap=idx_sb[:, col : col + cap // 16],
                        num_idxs=cap,
                        num_idxs_reg=cap,
                        elem_size=E3,
                        queue_num=qctr[0] % 4,
                    )
                    qctr[0] += 1
                s_g = s_tiles[g]
                ps = spanps_p.tile([128, E3], F32, tag="ps", name=f"ps_{g}")
                for blk in range(TB):
                    nc.tensor.matmul(
                        out=ps[:],
                        lhsT=s_g[:, blk, :],
                        rhs=gt[:, blk, :],
                        start=(blk == 0),
                        stop=(blk == TB - 1),
                    )
                feat = cf3[b] if t == "c" else kf3[b]
                nc.vector.tensor_copy(out=feat[:, gg, :, :], in_=ps[:])
                featT = cfT3[b] if t == "c" else kfT3[b]
                tp = tp_p.tile([128, HOPS, 128], F16, tag="tp", name=f"tpg_{g}")
                for hop in range(HOPS):
                    nc.tensor.transpose(
                        out=tp[:, hop, :], in_=feat[:, gg, hop, :], identity=ident[:]
                    )
                nc.vector.tensor_copy(
                    out=featT[:, :, gg * 128 : (gg + 1) * 128], in_=tp[:]
                )

            def do_att_incr(b, kk, acc):
                # part: six bank-aligned 128-f32 output regions at (hop*2+gg)
                # *128, then six transposed softmax row-sums at 768+. Every
                # matmul is its own start+stop accumulation group — PSUM
                # corrupts when several open groups share a bank — and the
                # cross-KB-block accumulation happens in SBUF (acc) instead.
                part = outps_p.tile([128, 1024], F32, tag="part", name=f"pt_{b}_{kk}")
                for hop in range(HOPS):
                    att = attps_p.tile(
                        [128, Lc], F32, tag="att", name=f"att_{b}_{kk}_{hop}"
                    )
                    nc.tensor.matmul(
                        out=att[:],
                        lhsT=kfT3[b][:, hop, kk * 128 : (kk + 1) * 128],
                        rhs=cfT3[b][:, hop, :],
                        start=True,
                        stop=True,
                    )
                    expT = expp.tile(
                        [128, Lc], F16, tag="expT", name=f"exp_{b}_{kk}_{hop}"
                    )
                    nc.scalar.activation(
                        out=expT[:],
                        in_=att[:],
                        func=mybir.ActivationFunctionType.Exp,
                        bias=ebias[:],
                    )
                    for gg in range(CONV_G):
                        r = hop * CONV_G + gg
                        # row-sums land transposed: rsum[l, 1] = expT[:, l].T @ 1
                        nc.tensor.matmul(
                            out=part[:, 768 + r : 769 + r],
                            lhsT=expT[:, gg * 128 : (gg + 1) * 128],
                            rhs=ones128[:],
                            start=True,
                            stop=True,
                        )
                        nc.tensor.matmul(
                            out=part[:, r * D : (r + 1) * D],
                            lhsT=expT[:, gg * 128 : (gg + 1) * 128],
                            rhs=kf3[b][:, kk, hop, :],
                            start=True,
                            stop=True,
                        )
                if kk == 0:
                    nc.vector.tensor_copy(out=acc[:], in_=part[:, 0:774])
                else:
                    nc.vector.tensor_add(out=acc[:], in0=acc[:], in1=part[:, 0:774])

            def finalize(b, acc):
                # per-gg so the first output DMA fires without waiting for
                # the second half of the softmax normalization
                for gg in range(CONV_G):
                    rinv = softp.tile(
                        [128, HOPS], F32, tag=f"ri{gg}", name=f"ri_{b}_{gg}"
                    )
                    nc.vector.reciprocal(
                        out=rinv[:],
                        in_=acc[:, 768:774].rearrange(
                            "p (h g) -> p h g", h=HOPS
                        )[:, :, gg],
                    )
                    sc = softp.tile(
                        [128, HOPS, D], F32, tag=f"sc{gg}", name=f"sc_{b}_{gg}"
                    )
                    nc.vector.tensor_tensor(
                        out=sc[:],
                        in0=acc[:, 0:768].rearrange(
                            "p (h g d) -> p h g d", h=HOPS, g=CONV_G
                        )[:, :, gg, :],
                        in1=rinv[:]
                        .rearrange("p (h o) -> p h o", o=1)
                        .to_broadcast([128, HOPS, D]),
                        op=mybir.AluOpType.mult,
                    )
                    nc.vector.tensor_add(
                        out=oacc[b][:, gg, :], in0=sc[:, 0, :], in1=sc[:, 1, :]
                    )
                    nc.vector.tensor_add(
                        out=oacc[b][:, gg, :], in0=oacc[b][:, gg, :], in1=sc[:, 2, :]
                    )
                    nc.sync.dma_start(
                        out=out_d[b, gg * 128 : (gg + 1) * 128, :],
                        in_=oacc[b][:, gg, :],
                    )

            accs = [featp.tile([128, 774], F32, name=f"acc_{b}") for b in range(BPC)]
            kb_seen = [0] * BPC
            for g, (t, b, gg) in enumerate(GROUPS):
                do_group(g)
                if t == "k":
                    do_att_incr(b, gg, accs[b])
                    kb_seen[b] += 1
                    if kb_seen[b] == KB_G:
                        finalize(b, accs[b])
    nc.compile()
    return nc


def assemble_output(results):
    out = np.empty((Lc, B, D), np.float32)
    for c in range(NCORES):
        o = results[c]["out"]
        for b in range(BPC):
            out[:, c * BPC + b, :] = o[b]
    return out


def kernel(conv_seqs, kb_arr, C, K):
    meta, in_maps = prepare(conv_seqs, kb_arr, C, K)
    nc = build_nc(meta)
    res = run_bass_kernel_spmd(nc, in_maps, list(range(NCORES))).results
    return assemble_output(res)



# revision 30
# speedup vs baseline: 1.1121x; 1.0007x over previous
"""Trainium2 kernel for nn_EncoderTreeSpanNN — split-table single-row gathers.

Design (final tuned version):
- Tables stored as [V, 3*D] f16 (hop-interleaved rows, 768B); each gather
  descriptor fetches only the needed row. Each group's 1024 tokens split
  into EXACTLY 512 lo + 512 hi via overlapping table views (lo=tab[0:32768]
  idx=t, hi=tab[17232:50000] idx=t-17232, both int16-safe); tokens in the
  overlap band are assigned to whichever side balances the split. Zero
  padding, zero memsets, 8 gather blocks per group.
- 24 gather calls round-robin the 4 SWDGE queues starting at queue 1 (the
  library-forcing dummy occupies queue 0, so the first real call is not
  stuck behind its drain). One dedicated gather tile per group: gathers
  never throttle on compute consuming earlier groups.
- All 12 selection matrices (slot -> span routing via is_equal against an
  uploaded iota) are built on DVE during the ~10us GPSIMD library-overlay
  load when DVE is otherwise idle; span reduction is 8 accumulating
  matmuls per group.
- Attention is computed transposed (attT[k,l] = kfT^T cfT) incrementally
  as each KB group's features land: exp (scaled by 2^-4 for f16 range),
  row-sum via ones-vector matmul, and the output matmuls accumulate into
  per-(hop,gg) PSUM regions with per-kk SBUF accumulation. Only the last
  KB block's chain remains in the tail.
"""

import sys

sys.path.insert(0, "/opt/trn_rl_repo")

import numpy as np

import concourse.bacc as bacc
import concourse.tile as tile
from concourse import mybir
from concourse.bass_utils import run_bass_kernel_spmd

# problem constants
V, D, HOPS = 50000, 128, 3
B, Lc, Mc = 16, 256, 8
Lk, Mk = 512, 8
NCORES = 8
BPC = B // NCORES
E3 = HOPS * D  # 384 elems per row (hop-interleaved)
LO_MAX = 32768  # lo view = tab[0:32768], idx = t (int16 max)
HI_BASE = 17232  # hi view = tab[17232:50000], idx = t-17232 (max 32767)
CONV_G = Lc // 128  # 2
KB_G = Lk // 128  # 4
TB = 8  # gather blocks per group (4 lo + 4 hi, exactly 512 tokens each)
NG_PER_B = CONV_G + KB_G  # 6
NG = BPC * NG_PER_B  # 12

F32 = mybir.dt.float32
F16 = mybir.dt.float16
I16 = mybir.dt.int16

EXP_BIAS = float(-4.0 * np.log(2.0))  # exp(att)*2^-4 keeps row-sums in f16 range

# per-core group list: per batch, conv groups then kb groups
GROUPS = []
for _b in range(BPC):
    for _gg in range(CONV_G):
        GROUPS.append(("c", _b, _gg))
    for _gg in range(KB_G):
        GROUPS.append(("k", _b, _gg))


def _pack_idx(flat):
    """[n] int16 -> [128, n//16] dma_gather index layout (8 replicas x 16)."""
    n = flat.shape[0]
    return np.tile(flat.reshape(n // 16, 16).T.astype(np.int16), (8, 1))


def prepare(conv_seqs, kb_arr, C, K):
    conv_seqs = np.asarray(conv_seqs)
    kb_arr = np.asarray(kb_arr)

    def row_table(T):
        # [HOPS, V, D] -> [V, HOPS*D] f16 (hop-interleaved rows)
        return (
            np.transpose(np.asarray(T, np.float32), (1, 0, 2))
            .reshape(V, E3)
            .astype(np.float16)
        )

    tab_c = row_table(C)
    tab_k = row_table(K)

    # per (core, group) split into exactly 512 lo + 512 hi tokens: the two
    # table views overlap on rows [17232, 32768), so tokens in that band can
    # be assigned to either side to balance the counts exactly (zero padding,
    # 8 gather blocks per group instead of 10)
    sides = {}
    for c in range(NCORES):
        for g, (t, b, gg) in enumerate(GROUPS):
            seqs = conv_seqs if t == "c" else kb_arr
            arr = seqs[c * BPC + b, gg * 128 : (gg + 1) * 128, :]  # [128, M]
            toks = arr.reshape(-1).astype(np.int64)  # position p*M+m -> span p
            spans = np.repeat(np.arange(128), arr.shape[1])
            mand_lo = toks < HI_BASE
            flex = (toks >= HI_BASE) & (toks < LO_MAX)
            need = 512 - int(mand_lo.sum())
            assert 0 <= need <= int(flex.sum()), "overlap band can't balance"
            to_lo = mand_lo.copy()
            to_lo[np.nonzero(flex)[0][:need]] = True
            sides[(c, g, 0)] = (toks[to_lo].astype(np.int16), spans[to_lo])
            sides[(c, g, 1)] = (
                (toks[~to_lo] - HI_BASE).astype(np.int16),
                spans[~to_lo],
            )
    cap = 512

    in_maps = []
    for c in range(NCORES):
        idx_all = np.empty((128, NG * 2 * (cap // 16)), np.int16)
        seg_all = np.full((128, NG, TB), -1.0, np.float32)
        for g in range(NG):
            for side in range(2):
                sidx, sspan = sides[(c, g, side)]
                n = len(sidx)
                assert n == cap
                col = (g * 2 + side) * (cap // 16)
                idx_all[:, col : col + cap // 16] = _pack_idx(sidx)
                # segment ids: slot i (block i//128, partition i%128) -> span
                blk = side * 4 + np.arange(n) // 128
                part = np.arange(n) % 128
                seg_all[part, g, blk] = sspan
        in_maps.append(
            {
                "tab_c": tab_c,
                "tab_k": tab_k,
                "idx_all": idx_all,
                "seg_all": seg_all,
                "ident": np.eye(128, dtype=np.float16),
                "iota": np.broadcast_to(
                    np.arange(128, dtype=np.float32), (128, 128)
                ).copy(),
            }
        )
    return {"cap": cap}, in_maps


def build_nc(meta):
    cap = meta["cap"]
    nc = bacc.Bacc(num_swdge_queues=4)
    tab_c = nc.declare_dram_parameter("tab_c", [V, E3], F16, False)
    tab_k = nc.declare_dram_parameter("tab_k", [V, E3], F16, False)
    idx_d = nc.declare_dram_parameter("idx_all", [128, NG * 2 * (cap // 16)], I16, False)
    seg_d = nc.declare_dram_parameter("seg_all", [128, NG, TB], F32, False)
    ident_d = nc.declare_dram_parameter("ident", [128, 128], F16, False)
    iota_d = nc.declare_dram_parameter("iota", [128, 128], F32, False)
    out_d = nc.declare_dram_parameter("out", [BPC, Lc, D], F32, True)

    tab_lo = {"c": tab_c[0:LO_MAX], "k": tab_k[0:LO_MAX]}
    tab_hi = {"c": tab_c[HI_BASE:V], "k": tab_k[HI_BASE:V]}

    with tile.TileContext(nc) as tc:
        with (
            tc.tile_pool(name="constp", bufs=1) as constp,
            tc.tile_pool(name="gp", bufs=1) as gp,
            tc.tile_pool(name="sp", bufs=1) as sp,
            tc.tile_pool(name="featp", bufs=1) as featp,
            tc.tile_pool(name="expp", bufs=3) as expp,
            tc.tile_pool(name="softp", bufs=4) as softp,
            tc.tile_pool(name="spanps_p", bufs=2, space="PSUM") as spanps_p,
            tc.tile_pool(name="attps_p", bufs=2, space="PSUM") as attps_p,
            tc.tile_pool(name="tp_p", bufs=2, space="PSUM") as tp_p,
            tc.tile_pool(name="outps_p", bufs=1, space="PSUM") as outps_p,
        ):
            idx_sb = constp.tile([128, NG * 2 * (cap // 16)], I16)
            nc.sync.dma_start(out=idx_sb[:], in_=idx_d[:])
            seg_sb = constp.tile([128, NG, TB], F32)
            nc.sync.dma_start(out=seg_sb[:], in_=seg_d[:])
            ident = constp.tile([128, 128], F16)
            nc.sync.dma_start(out=ident[:], in_=ident_d[:])
            iota = constp.tile([128, 128], F32)
            nc.sync.dma_start(out=iota[:], in_=iota_d[:])
            ones128 = constp.tile([128, 1], F16)
            nc.vector.memset(ones128[:], 1.0)
            ebias = constp.tile([128, 1], F32)
            nc.vector.memset(ebias[:], EXP_BIAS)

            cf3 = [
                featp.tile([128, CONV_G, HOPS, D], F16, name=f"cf3_{b}")
                for b in range(BPC)
            ]
            kf3 = [
                featp.tile([128, KB_G, HOPS, D], F16, name=f"kf3_{b}")
                for b in range(BPC)
            ]
            cfT3 = [
                featp.tile([128, HOPS, Lc], F16, name=f"cfT3_{b}") for b in range(BPC)
            ]
            kfT3 = [
                featp.tile([128, HOPS, Lk], F16, name=f"kfT3_{b}") for b in range(BPC)
            ]
            oacc = [
                featp.tile([128, CONV_G, D], F32, name=f"oacc_{b}") for b in range(BPC)
            ]

            qctr = [0]  # no dummy: the first real gather triggers the library load

            # selection matrices prebuilt during the GPSIMD library-load
            # window (DVE is otherwise idle then)
            s_tiles = []
            for g in range(NG):
                s_g = sp.tile([128, TB, 128], F16, name=f"S_{g}")
                nc.vector.tensor_tensor(
                    out=s_g[:],
                    in0=seg_sb[:, g, :]
                    .rearrange("p (t o) -> p t o", o=1)
                    .to_broadcast([128, TB, 128]),
                    in1=iota[:]
                    .rearrange("p (o d) -> p o d", o=1)
                    .to_broadcast([128, TB, 128]),
                    op=mybir.AluOpType.is_equal,
                )
                s_tiles.append(s_g)

            def do_group(g):
                t, b, gg = GROUPS[g]
                gt = gp.tile([128, TB, E3], F16, tag=f"gt_{g}", name=f"gt_{g}")
                for side, tabs in ((0, tab_lo), (1, tab_hi)):
                    col = (g * 2 + side) * (cap // 16)
                    nc.gpsimd.dma_gather(
                        out_ap=gt[:, side * 4 : side * 4 + 4, :],
                        in_ap=tabs[t][:],
                        idxs_ap=idx_sb[:, col : col + cap // 16],
                        num_idxs=cap,
                        num_idxs_reg=cap,
                        elem_size=E3,
                        queue_num=qctr[0] % 4,
                    )
                    qctr[0] += 1
                s_g = s_tiles[g]
                ps = spanps_p.tile([128, E3], F32, tag="ps", name=f"ps_{g}")
                for blk in range(TB):
                    nc.tensor.matmul(
                        out=ps[:],
                        lhsT=s_g[:, blk, :],
                        rhs=gt[:, blk, :],
                        start=(blk == 0),
                        stop=(blk == TB - 1),
                    )
                feat = cf3[b] if t == "c" else kf3[b]
                nc.vector.tensor_copy(out=feat[:, gg, :, :], in_=ps[:])
                featT = cfT3[b] if t == "c" else kfT3[b]
                tp = tp_p.tile([128, HOPS, 128], F16, tag="tp", name=f"tpg_{g}")
                for hop in range(HOPS):
                    nc.tensor.transpose(
                        out=tp[:, hop, :], in_=feat[:, gg, hop, :], identity=ident[:]
                    )
                nc.vector.tensor_copy(
                    out=featT[:, :, gg * 128 : (gg + 1) * 128], in_=tp[:]
                )

            def do_att_incr(b, kk, acc):
                # part: six bank-aligned 128-f32 output regions at (hop*2+gg)
                # *128, then six transposed softmax row-sums at 768+. Every
                # matmul is its own start+stop accumulation group — PSUM
                # corrupts when several open groups share a bank — and the
                # cross-KB-block accumulation happens in SBUF (acc) instead.
                part = outps_p.tile([128, 1024], F32, tag="part", name=f"pt_{b}_{kk}")
                for hop in range(HOPS):
                    att = attps_p.tile(
                        [128, Lc], F32, tag="att", name=f"att_{b}_{kk}_{hop}"
                    )
                    nc.tensor.matmul(
                        out=att[:],
                        lhsT=kfT3[b][:, hop, kk * 128 : (kk + 1) * 128],
                        rhs=cfT3[b][:, hop, :],
                        start=True,
                        stop=True,
                    )
                    expT = expp.tile(
                        [128, Lc], F16, tag="expT", name=f"exp_{b}_{kk}_{hop}"
                    )
                    nc.scalar.activation(
                        out=expT[:],
                        in_=att[:],
                        func=mybir.ActivationFunctionType.Exp,
                        bias=ebias[:],
                    )
                    for gg in range(CONV_G):
                        r = hop * CONV_G + gg
                        # row-sums land transposed: rsum[l, 1] = expT[:, l].T @ 1
                        nc.tensor.matmul(
                            out=part[:, 768 + r : 769 + r],
                            lhsT=expT[:, gg * 128 : (gg + 1) * 128],
                            rhs=ones128[:],
                            start=True,
                            stop=True,
                        )
                        nc.tensor.matmul(
                            out=part[:, r * D : (r + 1) * D],
                            lhsT=expT[:, gg * 128 : (gg + 1) * 128],
                            rhs=kf3[b][:, kk, hop, :],
                            start=True,
                            stop=True,
                        )
                if kk == 0:
                    nc.vector.tensor_copy(out=acc[:], in_=part[:, 0:774])
                else:
                    nc.vector.tensor_add(out=acc[:], in0=acc[:], in1=part[:, 0:774])

            def finalize(b, acc):
                # per-gg so the first output DMA fires without waiting for
                # the second half of the softmax normalization
                for gg in range(CONV_G):
                    rinv = softp.tile(
                        [128, HOPS], F32, tag=f"ri{gg}", name=f"ri_{b}_{gg}"
                    )
                    nc.vector.reciprocal(
                        out=rinv[:],
                        in_=acc[:, 768:774].rearrange(
                            "p (h g) -> p h g", h=HOPS
                        )[:, :, gg],
                    )
                    sc = softp.tile(
                        [128, HOPS, D], F32, tag=f"sc{gg}", name=f"sc_{b}_{gg}"
                    )
                    nc.vector.tensor_tensor(
                        out=sc[:],
                        in0=acc[:, 0:768].rearrange(
                            "p (h g d) -> p h g d", h=HOPS, g=CONV_G
                        )[:, :, gg, :],
                        in1=rinv[:]
                        .rearrange("p (h o) -> p h o", o=1)
                        .to_broadcast([128, HOPS, D]),
                        op=mybir.AluOpType.mult,
                    )
                    nc.vector.tensor_add(
                        out=oacc[b][:, gg, :], in0=sc[:, 0, :], in1=sc[:, 1, :]
                    )
                    nc.vector.tensor_add(
                        out=oacc[b][:, gg, :], in0=oacc[b][:, gg, :], in1=sc[:, 2, :]
                    )
                    nc.sync.dma_start(
                        out=out_d[b, gg * 128 : (gg + 1) * 128, :],
                        in_=oacc[b][:, gg, :],
                    )

            accs = [featp.tile([128, 774], F32, name=f"acc_{b}") for b in range(BPC)]
            kb_seen = [0] * BPC
            for g, (t, b, gg) in enumerate(GROUPS):
                do_group(g)
                if t == "k":
                    do_att_incr(b, gg, accs[b])
                    kb_seen[b] += 1
                    if kb_seen[b] == KB_G:
                        finalize(b, accs[b])
    nc.compile()
    return nc


def assemble_output(results):
    out = np.empty((Lc, B, D), np.float32)
    for c in range(NCORES):
        o = results[c]["out"]
        for b in range(BPC):
            out[:, c * BPC + b, :] = o[b]
    return out


def kernel(conv_seqs, kb_arr, C, K):
    meta, in_maps = prepare(conv_seqs, kb_arr, C, K)
    nc = build_nc(meta)
    res = run_bass_kernel_spmd(nc, in_maps, list(range(NCORES))).results
    return assemble_output(res)



# revision 31
# speedup vs baseline: 1.1194x; 1.0066x over previous
"""Trainium2 kernel for nn_EncoderTreeSpanNN — split-table single-row gathers.

Design (final tuned version):
- Tables stored as [V, 3*D] f16 (hop-interleaved rows, 768B); each gather
  descriptor fetches only the needed row. Each group's 1024 tokens split
  into EXACTLY 512 lo + 512 hi via overlapping table views (lo=tab[0:32768]
  idx=t, hi=tab[17232:50000] idx=t-17232, both int16-safe); tokens in the
  overlap band are assigned to whichever side balances the split. Zero
  padding, zero memsets, 8 gather blocks per group.
- 24 gather calls round-robin the 4 SWDGE queues starting at queue 1 (the
  library-forcing dummy occupies queue 0, so the first real call is not
  stuck behind its drain). One dedicated gather tile per group: gathers
  never throttle on compute consuming earlier groups.
- All 12 selection matrices (slot -> span routing via is_equal against an
  uploaded iota) are built on DVE during the ~10us GPSIMD library-overlay
  load when DVE is otherwise idle; span reduction is 8 accumulating
  matmuls per group.
- Attention is computed transposed (attT[k,l] = kfT^T cfT) incrementally
  as each KB group's features land: exp (scaled by 2^-4 for f16 range),
  row-sum via ones-vector matmul, and the output matmuls accumulate into
  per-(hop,gg) PSUM regions with per-kk SBUF accumulation. Only the last
  KB block's chain remains in the tail.
"""

import sys

sys.path.insert(0, "/opt/trn_rl_repo")

import numpy as np

import concourse.bacc as bacc
import concourse.tile as tile
from concourse import mybir
from concourse.bass_utils import run_bass_kernel_spmd

# problem constants
V, D, HOPS = 50000, 128, 3
B, Lc, Mc = 16, 256, 8
Lk, Mk = 512, 8
NCORES = 8
BPC = B // NCORES
E3 = HOPS * D  # 384 elems per row (hop-interleaved)
LO_MAX = 32768  # lo view = tab[0:32768], idx = t (int16 max)
HI_BASE = 17232  # hi view = tab[17232:50000], idx = t-17232 (max 32767)
CONV_G = Lc // 128  # 2
KB_G = Lk // 128  # 4
TB = 8  # gather blocks per group (4 lo + 4 hi, exactly 512 tokens each)
NG_PER_B = CONV_G + KB_G  # 6
NG = BPC * NG_PER_B  # 12

F32 = mybir.dt.float32
F16 = mybir.dt.float16
I16 = mybir.dt.int16

EXP_BIAS = float(-4.0 * np.log(2.0))  # exp(att)*2^-4 keeps row-sums in f16 range

# per-core group list: per batch, conv groups then kb groups
GROUPS = []
for _b in range(BPC):
    for _gg in range(CONV_G):
        GROUPS.append(("c", _b, _gg))
    for _gg in range(KB_G):
        GROUPS.append(("k", _b, _gg))


def _pack_idx(flat):
    """[n] int16 -> [128, n//16] dma_gather index layout (8 replicas x 16)."""
    n = flat.shape[0]
    return np.tile(flat.reshape(n // 16, 16).T.astype(np.int16), (8, 1))


def prepare(conv_seqs, kb_arr, C, K):
    conv_seqs = np.asarray(conv_seqs)
    kb_arr = np.asarray(kb_arr)

    def row_table(T):
        # [HOPS, V, D] -> [V, HOPS*D] f16 (hop-interleaved rows)
        return (
            np.transpose(np.asarray(T, np.float32), (1, 0, 2))
            .reshape(V, E3)
            .astype(np.float16)
        )

    tab_c = row_table(C)
    tab_k = row_table(K)

    # per (core, group) split into exactly 512 lo + 512 hi tokens: the two
    # table views overlap on rows [17232, 32768), so tokens in that band can
    # be assigned to either side to balance the counts exactly (zero padding,
    # 8 gather blocks per group instead of 10)
    sides = {}
    for c in range(NCORES):
        for g, (t, b, gg) in enumerate(GROUPS):
            seqs = conv_seqs if t == "c" else kb_arr
            arr = seqs[c * BPC + b, gg * 128 : (gg + 1) * 128, :]  # [128, M]
            toks = arr.reshape(-1).astype(np.int64)  # position p*M+m -> span p
            spans = np.repeat(np.arange(128), arr.shape[1])
            mand_lo = toks < HI_BASE
            flex = (toks >= HI_BASE) & (toks < LO_MAX)
            need = 512 - int(mand_lo.sum())
            assert 0 <= need <= int(flex.sum()), "overlap band can't balance"
            to_lo = mand_lo.copy()
            to_lo[np.nonzero(flex)[0][:need]] = True
            sides[(c, g, 0)] = (toks[to_lo].astype(np.int16), spans[to_lo])
            sides[(c, g, 1)] = (
                (toks[~to_lo] - HI_BASE).astype(np.int16),
                spans[~to_lo],
            )
    cap = 512

    in_maps = []
    for c in range(NCORES):
        idx_all = np.empty((128, NG * 2 * (cap // 16)), np.int16)
        seg_all = np.full((128, NG, TB), -1.0, np.float32)
        for g in range(NG):
            for side in range(2):
                sidx, sspan = sides[(c, g, side)]
                n = len(sidx)
                assert n == cap
                col = (g * 2 + side) * (cap // 16)
                idx_all[:, col : col + cap // 16] = _pack_idx(sidx)
                # segment ids: slot i (block i//128, partition i%128) -> span
                blk = side * 4 + np.arange(n) // 128
                part = np.arange(n) % 128
                seg_all[part, g, blk] = sspan
        in_maps.append(
            {
                "tab_c": tab_c,
                "tab_k": tab_k,
                "idx_all": idx_all,
                "seg_all": seg_all,
                "ident": np.eye(128, dtype=np.float16),
                "iota": np.broadcast_to(
                    np.arange(128, dtype=np.float32), (128, 128)
                ).copy(),
            }
        )
    return {"cap": cap}, in_maps


def build_nc(meta):
    cap = meta["cap"]
    nc = bacc.Bacc(num_swdge_queues=4)
    tab_c = nc.declare_dram_parameter("tab_c", [V, E3], F16, False)
    tab_k = nc.declare_dram_parameter("tab_k", [V, E3], F16, False)
    idx_d = nc.declare_dram_parameter("idx_all", [128, NG * 2 * (cap // 16)], I16, False)
    seg_d = nc.declare_dram_parameter("seg_all", [128, NG, TB], F32, False)
    ident_d = nc.declare_dram_parameter("ident", [128, 128], F16, False)
    iota_d = nc.declare_dram_parameter("iota", [128, 128], F32, False)
    out_d = nc.declare_dram_parameter("out", [BPC, Lc, D], F32, True)

    tab_lo = {"c": tab_c[0:LO_MAX], "k": tab_k[0:LO_MAX]}
    tab_hi = {"c": tab_c[HI_BASE:V], "k": tab_k[HI_BASE:V]}

    with tile.TileContext(nc) as tc:
        with (
            tc.tile_pool(name="constp", bufs=1) as constp,
            tc.tile_pool(name="gp", bufs=1) as gp,
            tc.tile_pool(name="sp", bufs=1) as sp,
            tc.tile_pool(name="featp", bufs=1) as featp,
            tc.tile_pool(name="expp", bufs=3) as expp,
            tc.tile_pool(name="softp", bufs=4) as softp,
            tc.tile_pool(name="spanps_p", bufs=2, space="PSUM") as spanps_p,
            tc.tile_pool(name="attps_p", bufs=2, space="PSUM") as attps_p,
            tc.tile_pool(name="tp_p", bufs=2, space="PSUM") as tp_p,
            tc.tile_pool(name="outps_p", bufs=1, space="PSUM") as outps_p,
        ):
            # tiny dummy gather first: forces the GPSIMD library load (~9us
            # of DMA residency) to start before the input uploads
            dummy_idx = constp.tile([128, 1], I16)
            nc.vector.memset(dummy_idx[:], 0)
            dummy_out = constp.tile([128, 1, E3], F16)
            nc.gpsimd.dma_gather(
                out_ap=dummy_out[:],
                in_ap=tab_c[0:LO_MAX],
                idxs_ap=dummy_idx[:],
                num_idxs=16,
                num_idxs_reg=16,
                elem_size=E3,
                queue_num=0,
            )
            idx_sb = constp.tile([128, NG * 2 * (cap // 16)], I16)
            nc.sync.dma_start(out=idx_sb[:], in_=idx_d[:])
            seg_sb = constp.tile([128, NG, TB], F32)
            nc.sync.dma_start(out=seg_sb[:], in_=seg_d[:])
            ident = constp.tile([128, 128], F16)
            nc.sync.dma_start(out=ident[:], in_=ident_d[:])
            iota = constp.tile([128, 128], F32)
            nc.sync.dma_start(out=iota[:], in_=iota_d[:])
            ones128 = constp.tile([128, 1], F16)
            nc.vector.memset(ones128[:], 1.0)
            ebias = constp.tile([128, 1], F32)
            nc.vector.memset(ebias[:], EXP_BIAS)

            cf3 = [
                featp.tile([128, CONV_G, HOPS, D], F16, name=f"cf3_{b}")
                for b in range(BPC)
            ]
            kf3 = [
                featp.tile([128, KB_G, HOPS, D], F16, name=f"kf3_{b}")
                for b in range(BPC)
            ]
            cfT3 = [
                featp.tile([128, HOPS, Lc], F16, name=f"cfT3_{b}") for b in range(BPC)
            ]
            kfT3 = [
                featp.tile([128, HOPS, Lk], F16, name=f"kfT3_{b}") for b in range(BPC)
            ]
            oacc = [
                featp.tile([128, CONV_G, D], F32, name=f"oacc_{b}") for b in range(BPC)
            ]

            qctr = [1]  # first real gather on q1: q0 is busy with the dummy

            # selection matrices prebuilt during the GPSIMD library-load
            # window (DVE is otherwise idle then)
            s_tiles = []
            for g in range(NG):
                s_g = sp.tile([128, TB, 128], F16, name=f"S_{g}")
                nc.vector.tensor_tensor(
                    out=s_g[:],
                    in0=seg_sb[:, g, :]
                    .rearrange("p (t o) -> p t o", o=1)
                    .to_broadcast([128, TB, 128]),
                    in1=iota[:]
                    .rearrange("p (o d) -> p o d", o=1)
                    .to_broadcast([128, TB, 128]),
                    op=mybir.AluOpType.is_equal,
                )
                s_tiles.append(s_g)

            def do_group(g):
                t, b, gg = GROUPS[g]
                gt = gp.tile([128, TB, E3], F16, tag=f"gt_{g}", name=f"gt_{g}")
                # group 0 primes the pipeline with four quarter-calls (one per
                # queue): each retires in ~2.6us instead of ~5, so all four
                # queues are dispatched ~2.5us sooner (the in-order sequencer
                # blocks on each call's retire while its queue is busy)
                halves = 2 if g == 0 else 1
                for side, tabs in ((0, tab_lo), (1, tab_hi)):
                    col = (g * 2 + side) * (cap // 16)
                    n = cap // halves
                    for h in range(halves):
                        nc.gpsimd.dma_gather(
                            out_ap=gt[
                                :,
                                side * 4 + h * (4 // halves) : side * 4
                                + (h + 1) * (4 // halves),
                                :,
                            ],
                            in_ap=tabs[t][:],
                            idxs_ap=idx_sb[
                                :, col + h * (n // 16) : col + (h + 1) * (n // 16)
                            ],
                            num_idxs=n,
                            num_idxs_reg=n,
                            elem_size=E3,
                            queue_num=qctr[0] % 4,
                        )
                        qctr[0] += 1
                s_g = s_tiles[g]
                ps = spanps_p.tile([128, E3], F32, tag="ps", name=f"ps_{g}")
                for blk in range(TB):
                    nc.tensor.matmul(
                        out=ps[:],
                        lhsT=s_g[:, blk, :],
                        rhs=gt[:, blk, :],
                        start=(blk == 0),
                        stop=(blk == TB - 1),
                    )
                feat = cf3[b] if t == "c" else kf3[b]
                nc.vector.tensor_copy(out=feat[:, gg, :, :], in_=ps[:])
                featT = cfT3[b] if t == "c" else kfT3[b]
                tp = tp_p.tile([128, HOPS, 128], F16, tag="tp", name=f"tpg_{g}")
                for hop in range(HOPS):
                    nc.tensor.transpose(
                        out=tp[:, hop, :], in_=feat[:, gg, hop, :], identity=ident[:]
                    )
                nc.vector.tensor_copy(
                    out=featT[:, :, gg * 128 : (gg + 1) * 128], in_=tp[:]
                )

            def do_att_incr(b, kk, acc):
                # part: six bank-aligned 128-f32 output regions at (hop*2+gg)
                # *128, then six transposed softmax row-sums at 768+. Every
                # matmul is its own start+stop accumulation group — PSUM
                # corrupts when several open groups share a bank — and the
                # cross-KB-block accumulation happens in SBUF (acc) instead.
                part = outps_p.tile([128, 1024], F32, tag="part", name=f"pt_{b}_{kk}")
                for hop in range(HOPS):
                    att = attps_p.tile(
                        [128, Lc], F32, tag="att", name=f"att_{b}_{kk}_{hop}"
                    )
                    nc.tensor.matmul(
                        out=att[:],
                        lhsT=kfT3[b][:, hop, kk * 128 : (kk + 1) * 128],
                        rhs=cfT3[b][:, hop, :],
                        start=True,
                        stop=True,
                    )
                    expT = expp.tile(
                        [128, Lc], F16, tag="expT", name=f"exp_{b}_{kk}_{hop}"
                    )
                    nc.scalar.activation(
                        out=expT[:],
                        in_=att[:],
                        func=mybir.ActivationFunctionType.Exp,
                        bias=ebias[:],
                    )
                    for gg in range(CONV_G):
                        r = hop * CONV_G + gg
                        # row-sums land transposed: rsum[l, 1] = expT[:, l].T @ 1
                        nc.tensor.matmul(
                            out=part[:, 768 + r : 769 + r],
                            lhsT=expT[:, gg * 128 : (gg + 1) * 128],
                            rhs=ones128[:],
                            start=True,
                            stop=True,
                        )
                        nc.tensor.matmul(
                            out=part[:, r * D : (r + 1) * D],
                            lhsT=expT[:, gg * 128 : (gg + 1) * 128],
                            rhs=kf3[b][:, kk, hop, :],
                            start=True,
                            stop=True,
                        )
                if kk == 0:
                    nc.vector.tensor_copy(out=acc[:], in_=part[:, 0:774])
                else:
                    nc.vector.tensor_add(out=acc[:], in0=acc[:], in1=part[:, 0:774])

            def finalize(b, acc):
                # per-gg so the first output DMA fires without waiting for
                # the second half of the softmax normalization
                for gg in range(CONV_G):
                    rinv = softp.tile(
                        [128, HOPS], F32, tag=f"ri{gg}", name=f"ri_{b}_{gg}"
                    )
                    nc.vector.reciprocal(
                        out=rinv[:],
                        in_=acc[:, 768:774].rearrange(
                            "p (h g) -> p h g", h=HOPS
                        )[:, :, gg],
                    )
                    sc = softp.tile(
                        [128, HOPS, D], F32, tag=f"sc{gg}", name=f"sc_{b}_{gg}"
                    )
                    nc.vector.tensor_tensor(
                        out=sc[:],
                        in0=acc[:, 0:768].rearrange(
                            "p (h g d) -> p h g d", h=HOPS, g=CONV_G
                        )[:, :, gg, :],
                        in1=rinv[:]
                        .rearrange("p (h o) -> p h o", o=1)
                        .to_broadcast([128, HOPS, D]),
                        op=mybir.AluOpType.mult,
                    )
                    nc.vector.tensor_add(
                        out=oacc[b][:, gg, :], in0=sc[:, 0, :], in1=sc[:, 1, :]
                    )
                    nc.vector.tensor_add(
                        out=oacc[b][:, gg, :], in0=oacc[b][:, gg, :], in1=sc[:, 2, :]
                    )
                    nc.sync.dma_start(
                        out=out_d[b, gg * 128 : (gg + 1) * 128, :],
                        in_=oacc[b][:, gg, :],
                    )

            accs = [featp.tile([128, 774], F32, name=f"acc_{b}") for b in range(BPC)]
            kb_seen = [0] * BPC
            for g, (t, b, gg) in enumerate(GROUPS):
                do_group(g)
                if t == "k":
                    do_att_incr(b, gg, accs[b])
                    kb_seen[b] += 1
                    if kb_seen[b] == KB_G:
                        finalize(b, accs[b])
    nc.compile()
    return nc


def assemble_output(results):
    out = np.empty((Lc, B, D), np.float32)
    for c in range(NCORES):
        o = results[c]["out"]
        for b in range(BPC):
            out[:, c * BPC + b, :] = o[b]
    return out


def kernel(conv_seqs, kb_arr, C, K):
    meta, in_maps = prepare(conv_seqs, kb_arr, C, K)
    nc = build_nc(meta)
    res = run_bass_kernel_spmd(nc, in_maps, list(range(NCORES))).results
    return assemble_output(res)

